# revision 3
# baseline (speedup 1.0000x reference)
"""Trainium2 Bass kernel for DilatedSpatialAttention.

Problem (hardcoded): B=16, H=W=32, C=256, heads=8, head_dim=32,
depthwise 3x3 conv with dilation 2 (SAME) applied to key and value,
then standard softmax attention per (batch, head) over S=H*W=1024.

Sharding: data-parallel over batch across 8 cores (2 batches/core).

Per-core per-batch dataflow:
  1. DMA q/k/v s-major [1024, 256] f32.
  2. PE-transpose to c-major [256, 1024]; K/V land in a zero-padded
     [128, 36, 36] layout so all 9 dilated taps become clean shifted APs.
  3. Depthwise conv on the PE: per 32-channel chunk, 9 accumulated
     matmuls with a diagonal kappa weight in a 32x32 array tile;
     the 4 chunks of a half run in 4 concurrent diagonal tiles.
     Bias is added during PSUM->SBUF evacuation (tensor_scalar_add).
  4. scoresT[k, q] = Kc^T Qc via row-tiled (K=32) matmuls, 2 heads packed.
  5. P = exp(scale * scoresT) on ScalarE, PSUM -> SBUF bf16.
  6. outT[d, q] (+ row 32 = softmax denom) = [V|1]^T P accumulated over k.
  7. PE-transpose outT back, normalize with per-partition reciprocal
     on VectorE, assemble [128, 256] output rows, DMA out.
"""

import numpy as np

B, H, W, C = 16, 32, 32, 256
HEADS = 8
HD = C // HEADS            # 32
KSZ, DIL = 3, 2
SCALE = float(HD) ** -0.5
NCORES = 8
BPC = B // NCORES          # batches per core
S = H * W                  # 1024
NKT = S // 128             # 8 k/s tiles
F32 = None                 # filled lazily (mybir types)
BF16 = None

_CACHE = {}


def _build(nc, tile, bass, mybir):
    from contextlib import ExitStack
    from concourse.masks import make_identity

    f32 = mybir.dt.float32
    bf16 = mybir.dt.bfloat16

    q_d = nc.dram_tensor("query", [BPC, S, C], f32, kind="ExternalInput")
    k_d = nc.dram_tensor("key_in", [BPC, S, C], f32, kind="ExternalInput")
    v_d = nc.dram_tensor("value", [BPC, S, C], f32, kind="ExternalInput")
    ck_d = nc.dram_tensor("conv_kernel", [KSZ * KSZ, C], f32, kind="ExternalInput")
    cb_d = nc.dram_tensor("conv_bias", [C], f32, kind="ExternalInput")
    out_d = nc.dram_tensor("out", [BPC, S, C], f32, kind="ExternalOutput")

    HP = H + 2 * DIL  # padded rows: 36
    WP = W + 2 * DIL  # padded cols: 36

    with ExitStack() as ctx:
        tc = ctx.enter_context(tile.TileContext(nc))
        const = ctx.enter_context(tc.tile_pool(name="const", bufs=1))
        sin = ctx.enter_context(tc.tile_pool(name="sin", bufs=8))
        xpad_p = ctx.enter_context(tc.tile_pool(name="xpad", bufs=6))
        qc_p = ctx.enter_context(tc.tile_pool(name="qc", bufs=4))
        kc_p = ctx.enter_context(tc.tile_pool(name="kc", bufs=4))
        vc_p = ctx.enter_context(tc.tile_pool(name="vc", bufs=4))
        vaug_p = ctx.enter_context(tc.tile_pool(name="vaug", bufs=16))
        p_p = ctx.enter_context(tc.tile_pool(name="pp", bufs=3))
        ot_p = ctx.enter_context(tc.tile_pool(name="ot", bufs=2))
        orow_p = ctx.enter_context(tc.tile_pool(name="orow", bufs=16))
        small_p = ctx.enter_context(tc.tile_pool(name="small", bufs=8))
        # PSUM: trans/conv pool (2 banks) + scores (2x2 banks) + accum (2 banks)
        ppp = ctx.enter_context(tc.tile_pool(name="ppp", bufs=2, space="PSUM"))
        sc_p = ctx.enter_context(tc.tile_pool(name="scp", bufs=2, space="PSUM"))
        acc_p = ctx.enter_context(tc.tile_pool(name="accp", bufs=2, space="PSUM"))

        # ---- constants ----
        ident = const.tile([128, 128], f32)
        make_identity(nc, ident[:])

        # diagonal conv weights: Wd[32r+i, half, tap, j] = kappa[tap, 128*half+32r+i] iff i==j
        wd = const.tile([128, 2, KSZ * KSZ, 32], f32)
        for half in range(2):
            for r in range(4):
                src = bass.AP(
                    ck_d, half * 128 + 32 * r,
                    [[0, 32], [C, KSZ * KSZ], [1, 32]],
                )
                nc.gpsimd.dma_start(out=wd[32 * r:32 * r + 32, half], in_=src)
                nc.gpsimd.affine_select(
                    out=wd[32 * r:32 * r + 32, half],
                    in_=wd[32 * r:32 * r + 32, half],
                    compare_op=mybir.AluOpType.is_equal,
                    fill=0.0,
                    base=0,
                    # keep where partition_local - j == 0
                    pattern=[[0, KSZ * KSZ], [-1, 32]],
                    channel_multiplier=1,
                )

        wdb = const.tile([128, 2, KSZ * KSZ, 32], bf16)
        nc.vector.tensor_copy(out=wdb[:], in_=wd[:])

        bias_c = const.tile([128, 2], f32)
        for half in range(2):
            nc.gpsimd.dma_start(
                out=bias_c[:, half:half + 1],
                in_=bass.AP(cb_d, half * 128, [[1, 128], [1, 1]]),
            )

        for b in range(BPC):
            # ================= prep phase =================
            qc = [qc_p.tile([128, S], bf16, tag="qc", name="qc") for _ in range(2)]
            kc = [kc_p.tile([128, S], bf16, tag="kc", name="kc") for _ in range(2)]
            vc = [vc_p.tile([128, S], f32, tag="vc", name="vc") for _ in range(2)]
            xpad = {}
            for tname in ("k", "v"):
                for half in range(2):
                    xp = xpad_p.tile([128, HP, WP], bf16, tag="xpad", name="xpad")
                    nc.gpsimd.memset(xp[:], 0.0)
                    xpad[(tname, half)] = xp

            for tname, dram in (("k", k_d), ("v", v_d), ("q", q_d)):
                for kt in range(NKT):
                    st = sin.tile([128, C], f32, tag="sin", name="st")
                    nc.sync.dma_start(
                        out=st[:], in_=dram[b, kt * 128:(kt + 1) * 128, :]
                    )
                    for half in range(2):
                        pt = ppp.tile([128, 512], f32, tag="pp", name="pt")
                        nc.tensor.transpose(
                            pt[:, 0:128], st[:, 128 * half:128 * half + 128],
                            ident[:],
                        )
                        if tname == "q":
                            nc.vector.tensor_copy(
                                out=qc[half][:, kt * 128:(kt + 1) * 128],
                                in_=pt[:, 0:128],
                            )
                        else:
                            # write [c, 4 rows, 32 cols] into padded interior
                            dst = xpad[(tname, half)][
                                :, DIL + 4 * kt:DIL + 4 * kt + 4, DIL:DIL + W
                            ]
                            nc.vector.tensor_copy(
                                out=dst,
                                in_=pt[:, 0:128].rearrange(
                                    "p (r w) -> p r w", w=W
                                ),
                            )

            # ---- depthwise conv on PE (diagonal 32x32 tiles) ----
            for tname in ("k", "v"):
                for half in range(2):
                    xp = xpad[(tname, half)]
                    for sb in range(2):  # s blocks of 512 (16 h-rows)
                        cp = ppp.tile([128, 512], f32, tag="pp", name="pt")
                        for tap in range(KSZ * KSZ):
                            dh, dw = divmod(tap, KSZ)
                            for r in range(4):
                                nc.tensor.matmul(
                                    out=cp[32 * r:32 * r + 32, :],
                                    lhsT=wdb[32 * r:32 * r + 32, half, tap, :],
                                    rhs=xp[
                                        32 * r:32 * r + 32,
                                        16 * sb + DIL * dh:16 * sb + DIL * dh + 16,
                                        DIL * dw:DIL * dw + W,
                                    ],
                                    start=(tap == 0),
                                    stop=(tap == KSZ * KSZ - 1),
                                    tile_position=(32 * r, 32 * r),
                                )
                        dstt = kc[half] if tname == "k" else vc[half]
                        nc.vector.tensor_scalar_add(
                            out=dstt[:, sb * 512:(sb + 1) * 512],
                            in0=cp[:],
                            scalar1=bias_c[:, half:half + 1],
                        )

            # ---- V back to s-major, augmented with ones column ----
            vaug = [vaug_p.tile([128, HEADS * (HD + 1)], bf16, tag="va", name="va")
                    for _ in range(NKT)]
            for kt in range(NKT):
                va3 = vaug[kt][:].rearrange("p (h x) -> p h x", x=HD + 1)
                nc.vector.memset(va3[:, :, HD:HD + 1], 1.0)
                for half in range(2):
                    pt = ppp.tile([128, 512], f32, tag="pp", name="pt")
                    nc.tensor.transpose(
                        pt[:, 0:128], vc[half][:, kt * 128:(kt + 1) * 128],
                        ident[:],
                    )
                    nc.vector.tensor_copy(
                        out=va3[:, 4 * half:4 * half + 4, 0:HD],
                        in_=pt[:, 0:128].rearrange("p (h d) -> p h d", d=HD),
                    )

            # ================= attention phase =================
            orow = [orow_p.tile([128, C], f32, tag="orow", name="orow") for _ in range(NKT)]
            for pair in range(4):
                half, hl = divmod(pair, 2)
                for qb in range(2):
                    acc = acc_p.tile([128, 512], f32, tag="acc", name="acc")
                    for kt in range(NKT):
                        sc = sc_p.tile([128, 2, 512], f32, tag="sc", name="sc")
                        for j in range(2):
                            base = 64 * hl + 32 * j
                            nc.tensor.matmul(
                                out=sc[:, j, :],
                                lhsT=kc[half][base:base + 32,
                                              kt * 128:(kt + 1) * 128],
                                rhs=qc[half][base:base + 32,
                                             qb * 512:(qb + 1) * 512],
                                start=True, stop=True,
                                tile_position=(base, 0),
                            )
                        p = p_p.tile([128, 2, 512], bf16, tag="p", name="p")
                        nc.scalar.activation(
                            out=p[:], in_=sc[:],
                            func=mybir.ActivationFunctionType.Exp,
                            scale=SCALE,
                        )
                        for j in range(2):
                            hglob = half * 4 + hl * 2 + j
                            nc.tensor.matmul(
                                out=acc[64 * j:64 * j + HD + 1, :],
                                lhsT=vaug[kt][:, (HD + 1) * hglob:
                                              (HD + 1) * hglob + HD + 1],
                                rhs=p[:, j, :],
                                start=(kt == 0), stop=(kt == NKT - 1),
                            )
                    # ---- normalize + emit ----
                    ot = ot_p.tile([128, 512], f32, tag="ot", name="ot")
                    nc.vector.tensor_copy(out=ot[0:97, :], in_=acc[0:97, :])
                    for u in range(4):
                        tp = ppp.tile([128, 512], f32, tag="pp", name="pt")
                        nc.tensor.transpose(
                            tp[:, 0:97], ot[0:97, u * 128:(u + 1) * 128],
                            ident[0:97, 0:97],
                        )
                        for j in range(2):
                            hglob = half * 4 + hl * 2 + j
                            rc = small_p.tile([128, 1], f32, tag="rc", name="rc")
                            nc.vector.reciprocal(
                                rc[:], tp[:, 64 * j + HD:64 * j + HD + 1]
                            )
                            nc.vector.tensor_scalar_mul(
                                out=orow[qb * 4 + u][:, HD * hglob:
                                                     HD * hglob + HD],
                                in0=tp[:, 64 * j:64 * j + HD],
                                scalar1=rc[:],
                            )
            for u in range(NKT):
                nc.sync.dma_start(
                    out=out_d[b, u * 128:(u + 1) * 128, :], in_=orow[u][:]
                )

    return nc


def _get_nc():
    if "nc" not in _CACHE:
        import concourse.bass as bass
        import concourse.tile as tile
        from concourse import bacc, mybir

        nc = bacc.Bacc("TRN2", target_bir_lowering=False, debug=False)
        _build(nc, tile, bass, mybir)
        nc.compile()
        _CACHE["nc"] = nc
    return _CACHE["nc"]


def kernel(**inputs):
    q = np.ascontiguousarray(
        np.asarray(inputs["query"], dtype=np.float32).reshape(B, S, C))
    k = np.ascontiguousarray(
        np.asarray(inputs["key_in"], dtype=np.float32).reshape(B, S, C))
    v = np.ascontiguousarray(
        np.asarray(inputs["value"], dtype=np.float32).reshape(B, S, C))
    ck = np.ascontiguousarray(
        np.asarray(inputs["conv_kernel"], dtype=np.float32).reshape(
            KSZ * KSZ, C))
    cb = np.ascontiguousarray(
        np.asarray(inputs["conv_bias"], dtype=np.float32).reshape(C))

    in_maps = []
    for i in range(NCORES):
        lo, hi = i * BPC, (i + 1) * BPC
        in_maps.append({
            "query": np.ascontiguousarray(q[lo:hi]),
            "key_in": np.ascontiguousarray(k[lo:hi]),
            "value": np.ascontiguousarray(v[lo:hi]),
            "conv_kernel": ck,
            "conv_bias": cb,
        })

    from concourse.bass_utils import run_bass_kernel_spmd

    nc = _get_nc()
    res = run_bass_kernel_spmd(
        nc, in_maps, core_ids=list(range(NCORES)),
        **_CACHE.get("run_kwargs", {}),
    )
    _CACHE["last_result"] = res
    out = np.concatenate([r["out"] for r in res.results], axis=0)
    return out.reshape(B, H, W, C)


# revision 5
# speedup vs baseline: 25.3872x; 25.3872x over previous
"""Trainium2 Bass kernel for DilatedSpatialAttention.

Problem (hardcoded): B=16, H=W=32, C=256, heads=8, head_dim=32,
depthwise 3x3 conv with dilation 2 (SAME) applied to key and value,
then standard softmax attention per (batch, head) over S=H*W=1024.

Sharding: data-parallel over batch across 8 cores (2 batches/core).

Per-core per-batch dataflow:
  1. DMA q/k/v s-major [1024, 256] f32.
  2. PE-transpose to c-major [256, 1024]; K/V land in a zero-padded
     [128, 36, 36] layout so all 9 dilated taps become clean shifted APs.
  3. Depthwise conv on the PE: per 32-channel chunk, 9 accumulated
     matmuls with a diagonal kappa weight in a 32x32 array tile;
     the 4 chunks of a half run in 4 concurrent diagonal tiles.
     Bias is added during PSUM->SBUF evacuation (tensor_scalar_add).
  4. scoresT[k, q] = Kc^T Qc via row-tiled (K=32) matmuls, 2 heads packed.
  5. P = exp(scale * scoresT) on ScalarE, PSUM -> SBUF bf16.
  6. outT[d, q] (+ row 32 = softmax denom) = [V|1]^T P accumulated over k.
  7. PE-transpose outT back, normalize with per-partition reciprocal
     on VectorE, assemble [128, 256] output rows, DMA out.
"""

import numpy as np

B, H, W, C = 16, 32, 32, 256
HEADS = 8
HD = C // HEADS            # 32
KSZ, DIL = 3, 2
SCALE = float(HD) ** -0.5
NCORES = 8
BPC = B // NCORES          # batches per core
S = H * W                  # 1024
NKT = S // 128             # 8 k/s tiles
F32 = None                 # filled lazily (mybir types)
BF16 = None

_CACHE = {}


def _build(nc, tile, bass, mybir, repeat=None):
    from contextlib import ExitStack
    from concourse.masks import make_identity

    f32 = mybir.dt.float32
    bf16 = mybir.dt.bfloat16

    q_d = nc.dram_tensor("query", [BPC, S, C], f32, kind="ExternalInput")
    k_d = nc.dram_tensor("key_in", [BPC, S, C], f32, kind="ExternalInput")
    v_d = nc.dram_tensor("value", [BPC, S, C], f32, kind="ExternalInput")
    ck_d = nc.dram_tensor("conv_kernel", [KSZ * KSZ, C], f32, kind="ExternalInput")
    cb_d = nc.dram_tensor("conv_bias", [C], f32, kind="ExternalInput")
    out_d = nc.dram_tensor("out", [BPC, S, C], f32, kind="ExternalOutput")

    HP = H + 2 * DIL  # padded rows: 36
    WP = W + 2 * DIL  # padded cols: 36

    with ExitStack() as ctx:
        tc = ctx.enter_context(tile.TileContext(nc))
        const = ctx.enter_context(tc.tile_pool(name="const", bufs=1))
        sin = ctx.enter_context(tc.tile_pool(name="sin", bufs=8))
        xpad_p = ctx.enter_context(tc.tile_pool(name="xpad", bufs=6))
        qc_p = ctx.enter_context(tc.tile_pool(name="qc", bufs=4))
        kc_p = ctx.enter_context(tc.tile_pool(name="kc", bufs=4))
        vc_p = ctx.enter_context(tc.tile_pool(name="vc", bufs=4))
        vaug_p = ctx.enter_context(tc.tile_pool(name="vaug", bufs=16))
        p_p = ctx.enter_context(tc.tile_pool(name="pp", bufs=3))
        ot_p = ctx.enter_context(tc.tile_pool(name="ot", bufs=2))
        orow_p = ctx.enter_context(tc.tile_pool(name="orow", bufs=16))
        small_p = ctx.enter_context(tc.tile_pool(name="small", bufs=8))
        # PSUM: trans/conv pool (2 banks) + scores (2x2 banks) + accum (2 banks)
        ppp = ctx.enter_context(tc.tile_pool(name="ppp", bufs=2, space="PSUM"))
        sc_p = ctx.enter_context(tc.tile_pool(name="scp", bufs=2, space="PSUM"))
        acc_p = ctx.enter_context(tc.tile_pool(name="accp", bufs=2, space="PSUM"))

        # ---- constants ----
        ident = const.tile([128, 128], f32)
        make_identity(nc, ident[:])

        # diagonal conv weights: Wd[32r+i, half, tap, j] = kappa[tap, 128*half+32r+i] iff i==j
        wd = const.tile([128, 2, KSZ * KSZ, 32], f32)
        for half in range(2):
            for r in range(4):
                src = bass.AP(
                    ck_d, half * 128 + 32 * r,
                    [[0, 32], [C, KSZ * KSZ], [1, 32]],
                )
                nc.gpsimd.dma_start(out=wd[32 * r:32 * r + 32, half], in_=src)
                nc.gpsimd.affine_select(
                    out=wd[32 * r:32 * r + 32, half],
                    in_=wd[32 * r:32 * r + 32, half],
                    compare_op=mybir.AluOpType.is_equal,
                    fill=0.0,
                    base=0,
                    # keep where partition_local - j == 0
                    pattern=[[0, KSZ * KSZ], [-1, 32]],
                    channel_multiplier=1,
                )

        wdb = const.tile([128, 2, KSZ * KSZ, 32], bf16)
        nc.vector.tensor_copy(out=wdb[:], in_=wd[:])

        bias_c = const.tile([128, 2], f32)
        for half in range(2):
            nc.gpsimd.dma_start(
                out=bias_c[:, half:half + 1],
                in_=bass.AP(cb_d, half * 128, [[1, 128], [1, 1]]),
            )

        rep_ctx = tc.For_i(0, repeat, 1) if repeat else None
        if rep_ctx is not None:
            ctx.enter_context(rep_ctx)
        for b in range(BPC):
            # ================= prep phase =================
            qc = [qc_p.tile([128, S], bf16, tag="qc", name="qc") for _ in range(2)]
            kc = [kc_p.tile([128, S], bf16, tag="kc", name="kc") for _ in range(2)]
            vc = [vc_p.tile([128, S], f32, tag="vc", name="vc") for _ in range(2)]
            xpad = {}
            for tname in ("k", "v"):
                for half in range(2):
                    xp = xpad_p.tile([128, HP, WP], bf16, tag="xpad", name="xpad")
                    nc.gpsimd.memset(xp[:], 0.0)
                    xpad[(tname, half)] = xp

            for tname, dram in (("k", k_d), ("v", v_d), ("q", q_d)):
                for kt in range(NKT):
                    st = sin.tile([128, C], f32, tag="sin", name="st")
                    nc.sync.dma_start(
                        out=st[:], in_=dram[b, kt * 128:(kt + 1) * 128, :]
                    )
                    for half in range(2):
                        pt = ppp.tile([128, 512], f32, tag="pp", name="pt")
                        nc.tensor.transpose(
                            pt[:, 0:128], st[:, 128 * half:128 * half + 128],
                            ident[:],
                        )
                        if tname == "q":
                            nc.vector.tensor_copy(
                                out=qc[half][:, kt * 128:(kt + 1) * 128],
                                in_=pt[:, 0:128],
                            )
                        else:
                            # write [c, 4 rows, 32 cols] into padded interior
                            dst = xpad[(tname, half)][
                                :, DIL + 4 * kt:DIL + 4 * kt + 4, DIL:DIL + W
                            ]
                            nc.vector.tensor_copy(
                                out=dst,
                                in_=pt[:, 0:128].rearrange(
                                    "p (r w) -> p r w", w=W
                                ),
                            )

            # ---- depthwise conv on PE (diagonal 32x32 tiles) ----
            for tname in ("k", "v"):
                for half in range(2):
                    xp = xpad[(tname, half)]
                    for sb in range(2):  # s blocks of 512 (16 h-rows)
                        cp = ppp.tile([128, 512], f32, tag="pp", name="pt")
                        for tap in range(KSZ * KSZ):
                            dh, dw = divmod(tap, KSZ)
                            for r in range(4):
                                nc.tensor.matmul(
                                    out=cp[32 * r:32 * r + 32, :],
                                    lhsT=wdb[32 * r:32 * r + 32, half, tap, :],
                                    rhs=xp[
                                        32 * r:32 * r + 32,
                                        16 * sb + DIL * dh:16 * sb + DIL * dh + 16,
                                        DIL * dw:DIL * dw + W,
                                    ],
                                    start=(tap == 0),
                                    stop=(tap == KSZ * KSZ - 1),
                                    tile_position=(32 * r, 32 * r),
                                )
                        dstt = kc[half] if tname == "k" else vc[half]
                        nc.vector.tensor_scalar_add(
                            out=dstt[:, sb * 512:(sb + 1) * 512],
                            in0=cp[:],
                            scalar1=bias_c[:, half:half + 1],
                        )

            # ---- V back to s-major, augmented with ones column ----
            vaug = [vaug_p.tile([128, HEADS * (HD + 1)], bf16, tag="va", name="va")
                    for _ in range(NKT)]
            for kt in range(NKT):
                va3 = vaug[kt][:].rearrange("p (h x) -> p h x", x=HD + 1)
                nc.vector.memset(va3[:, :, HD:HD + 1], 1.0)
                for half in range(2):
                    pt = ppp.tile([128, 512], f32, tag="pp", name="pt")
                    nc.tensor.transpose(
                        pt[:, 0:128], vc[half][:, kt * 128:(kt + 1) * 128],
                        ident[:],
                    )
                    nc.vector.tensor_copy(
                        out=va3[:, 4 * half:4 * half + 4, 0:HD],
                        in_=pt[:, 0:128].rearrange("p (h d) -> p h d", d=HD),
                    )

            # ================= attention phase =================
            orow = [orow_p.tile([128, C], f32, tag="orow", name="orow") for _ in range(NKT)]
            for pair in range(4):
                half, hl = divmod(pair, 2)
                for qb in range(2):
                    acc = acc_p.tile([128, 512], f32, tag="acc", name="acc")
                    for kt in range(NKT):
                        sc = sc_p.tile([128, 2, 512], f32, tag="sc", name="sc")
                        for j in range(2):
                            base = 64 * hl + 32 * j
                            nc.tensor.matmul(
                                out=sc[:, j, :],
                                lhsT=kc[half][base:base + 32,
                                              kt * 128:(kt + 1) * 128],
                                rhs=qc[half][base:base + 32,
                                             qb * 512:(qb + 1) * 512],
                                start=True, stop=True,
                                tile_position=(base, 0),
                            )
                        p = p_p.tile([128, 2, 512], bf16, tag="p", name="p")
                        nc.scalar.activation(
                            out=p[:], in_=sc[:],
                            func=mybir.ActivationFunctionType.Exp,
                            scale=SCALE,
                        )
                        for j in range(2):
                            hglob = half * 4 + hl * 2 + j
                            nc.tensor.matmul(
                                out=acc[64 * j:64 * j + HD + 1, :],
                                lhsT=vaug[kt][:, (HD + 1) * hglob:
                                              (HD + 1) * hglob + HD + 1],
                                rhs=p[:, j, :],
                                start=(kt == 0), stop=(kt == NKT - 1),
                                tile_position=(0, 64 * j),
                            )
                    # ---- normalize + emit ----
                    ot = ot_p.tile([128, 512], f32, tag="ot", name="ot")
                    nc.vector.tensor_copy(out=ot[0:97, :], in_=acc[0:97, :])
                    for u in range(4):
                        tp = ppp.tile([128, 512], f32, tag="pp", name="pt")
                        nc.tensor.transpose(
                            tp[:, 0:97], ot[0:97, u * 128:(u + 1) * 128],
                            ident[0:97, 0:97],
                        )
                        for j in range(2):
                            hglob = half * 4 + hl * 2 + j
                            rc = small_p.tile([128, 1], f32, tag="rc", name="rc")
                            nc.vector.reciprocal(
                                rc[:], tp[:, 64 * j + HD:64 * j + HD + 1]
                            )
                            nc.vector.tensor_scalar_mul(
                                out=orow[qb * 4 + u][:, HD * hglob:
                                                     HD * hglob + HD],
                                in0=tp[:, 64 * j:64 * j + HD],
                                scalar1=rc[:],
                            )
            for u in range(NKT):
                nc.sync.dma_start(
                    out=out_d[b, u * 128:(u + 1) * 128, :], in_=orow[u][:]
                )

    return nc


def _get_nc():
    if "nc" not in _CACHE:
        import concourse.bass as bass
        import concourse.tile as tile
        from concourse import bacc, mybir

        nc = bacc.Bacc("TRN2", target_bir_lowering=False, debug=False)
        _build(nc, tile, bass, mybir)
        nc.compile()
        _CACHE["nc"] = nc
    return _CACHE["nc"]


def kernel(**inputs):
    q = np.ascontiguousarray(
        np.asarray(inputs["query"], dtype=np.float32).reshape(B, S, C))
    k = np.ascontiguousarray(
        np.asarray(inputs["key_in"], dtype=np.float32).reshape(B, S, C))
    v = np.ascontiguousarray(
        np.asarray(inputs["value"], dtype=np.float32).reshape(B, S, C))
    ck = np.ascontiguousarray(
        np.asarray(inputs["conv_kernel"], dtype=np.float32).reshape(
            KSZ * KSZ, C))
    cb = np.ascontiguousarray(
        np.asarray(inputs["conv_bias"], dtype=np.float32).reshape(C))

    in_maps = []
    for i in range(NCORES):
        lo, hi = i * BPC, (i + 1) * BPC
        in_maps.append({
            "query": np.ascontiguousarray(q[lo:hi]),
            "key_in": np.ascontiguousarray(k[lo:hi]),
            "value": np.ascontiguousarray(v[lo:hi]),
            "conv_kernel": ck,
            "conv_bias": cb,
        })

    from concourse.bass_utils import run_bass_kernel_spmd

    nc = _get_nc()
    res = run_bass_kernel_spmd(
        nc, in_maps, core_ids=list(range(NCORES)),
        **_CACHE.get("run_kwargs", {}),
    )
    _CACHE["last_result"] = res
    out = np.concatenate([r["out"] for r in res.results], axis=0)
    return out.reshape(B, H, W, C)
